# revision 13
# baseline (speedup 1.0000x reference)
"""MeshConv (gnn_message_passing) Bass kernel for 8 trn2 NeuronCores.

out[b,o,v] = bias[o] + sum_k coeffs[k,:,o]^T feats_k[b,v,:]
  feats_0 = x^T (identity), feats_{1,2,3} = spmm(L/EW/NS, x)

Strategy: shard output vertices across cores (row-partitioned spmm).
Edges sorted by destination row into 128-edge chunks per 128-row tile.
Per chunk: dma_gather of x rows (1KB rows, int16 indices split lo/hi
around row 32768, 4 SWDGE queues in parallel), a one-hot
[edge, row_local]*val matrix built on DVE with one fused tensor_scalar,
and a PE matmul accumulating y_k[row, (b,c)] in PSUM.  y is transposed on
PE and hit with the per-operator coeffs (free dim 256 => full-rate f32r),
bias added on DVE, output written as [o, rows] slabs per batch.
"""

import sys

sys.path.insert(0, "/opt/trn_rl_repo")

import numpy as np

import concourse.bass as bass
import concourse.bacc as bacc
import concourse.tile as tile
import concourse.mybir as mybir
from concourse.bass_utils import run_bass_kernel_spmd
from concourse.masks import make_identity

NV = 40962
B = 4
C = 64
BC = B * C  # 256
NCORES = 8
NTILE = 328          # 128-row tiles, 328*128 = 41984 >= 40962
NVPAD = NTILE * 128
TPC = NTILE // NCORES  # 41 tiles per core
SPLIT = 32768        # int16 index split point
MAXCH = 8            # dma_gather limit: <=1024 indices per call
NQ = 4               # SWDGE queues

MM_MODE = "f32r"     # "f32r" (fast, ~3e-4 rel err) or "f32" (exact, slower)

_cache = {}


def _trunc_f32r(a):
    return (a.view(np.uint32) & np.uint32(0xFFFFF000)).view(np.float32)


def _prep_op(row, col, val):
    """Sort edges by row; per (tile, half) bucket into 128-slot chunks.

    Slot layout per tile: [C_lo chunks | C_hi chunks]; slot (j, p) holds the
    (j*128+p)-th edge of its half-group.  Returns per-[NTILE, 128, C] arrays
    idx (int16, relative to half base), rloc (f32), val (f32) plus (C_lo,
    C_hi).
    """
    row = np.asarray(row).astype(np.int64)
    col = np.asarray(col).astype(np.int64)
    val = np.asarray(val).astype(np.float32)
    order = np.argsort(row, kind="stable")
    row, col, val = row[order], col[order], val[order]
    tile_id = row >> 7
    ishi = (col >= SPLIT).astype(np.int64)

    halves = []
    for h in (0, 1):
        m = ishi == h
        r_h, c_h, v_h, t_h = row[m], col[m], val[m], tile_id[m]
        counts = np.bincount(t_h, minlength=NTILE)
        Ch = int(np.ceil(max(int(counts.max()), 1) / 128))
        slots = Ch * 128
        starts = np.zeros(NTILE, np.int64)
        starts[1:] = np.cumsum(counts)[:-1]
        pos = np.arange(len(r_h)) - starts[t_h]
        flat = t_h * slots + pos
        idxP = np.zeros(NTILE * slots, np.int16)
        rlocP = np.zeros(NTILE * slots, np.float32)
        valP = np.zeros(NTILE * slots, np.float32)
        idxP[flat] = (c_h - h * SPLIT).astype(np.int16)
        rlocP[flat] = (r_h & 127).astype(np.float32)
        valP[flat] = v_h
        # [NTILE, C, 128] -> [NTILE, 128, C]
        halves.append((
            Ch,
            idxP.reshape(NTILE, Ch, 128).transpose(0, 2, 1),
            rlocP.reshape(NTILE, Ch, 128).transpose(0, 2, 1),
            valP.reshape(NTILE, Ch, 128).transpose(0, 2, 1),
        ))
    (C_lo, i_lo, r_lo, v_lo), (C_hi, i_hi, r_hi, v_hi) = halves
    idxP = np.concatenate([i_lo, i_hi], axis=2)
    rlocP = np.concatenate([r_lo, r_hi], axis=2)
    valP = np.concatenate([v_lo, v_hi], axis=2)
    return (C_lo, C_hi), idxP, rlocP, valP


def _wrap16(arr):
    """[n] int16 (n%16==0) -> [128, n//16]: wrapped in 16 partitions,
    replicated for the 8 gpsimd cores."""
    n = arr.shape[-1]
    t16 = arr.reshape(-1, n // 16, 16)
    t16 = np.swapaxes(t16, -1, -2)  # [..., 16, n//16]
    return np.tile(t16, (1, 8, 1)) if arr.ndim > 1 else np.tile(t16[0], (8, 1))


def _calls(S_ops):
    """Static per-tile gather call list: (op_i, chunk_off, nchunks, is_hi)."""
    calls = []
    off = 0
    for opi, (C_lo, C_hi) in enumerate(S_ops):
        for h, Ch in ((0, C_lo), (1, C_hi)):
            a = 0
            while a < Ch:
                n = min(MAXCH, Ch - a)
                calls.append((opi, off + a, n, h))
                a += n
            off += Ch
    return calls


def _build(S_ops):
    """Build the per-core Bass program for ((C_L_lo,C_L_hi),(..E..),(..N..))."""
    STOT = sum(c for p in S_ops for c in p)
    f32 = mybir.dt.float32
    f32r = mybir.dt.float32r if MM_MODE == "f32r" else mybir.dt.float32

    nc = bacc.Bacc("TRN2", target_bir_lowering=False, debug=False,
                   num_devices=NCORES, num_swdge_queues=NQ)

    xg_d = nc.dram_tensor("xg", [NVPAD, BC], f32r, kind="ExternalInput")
    xTown_d = nc.dram_tensor("xTown", [TPC * 128, BC], f32,
                             kind="ExternalInput")
    idx_d = nc.dram_tensor("idx16", [128, TPC * STOT * 8], mybir.dt.int16,
                           kind="ExternalInput")
    rloc_d = nc.dram_tensor("rloc", [128, TPC * STOT], f32,
                            kind="ExternalInput")
    val_d = nc.dram_tensor("val", [128, TPC * STOT], f32,
                           kind="ExternalInput")
    iota_d = nc.dram_tensor("iota", [128, 128], f32, kind="ExternalInput")
    coef_d = nc.dram_tensor("coef", [64, 256], f32, kind="ExternalInput")
    bias_d = nc.dram_tensor("bias2", [128, 1], f32, kind="ExternalInput")
    out_d = nc.dram_tensor("out", [B, C, TPC * 128], f32,
                           kind="ExternalOutput")

    calls = _calls(S_ops)
    OPNAMES = ["L", "E", "N"]
    # chunk index ranges per op
    op_off = []
    o = 0
    for C_lo, C_hi in S_ops:
        op_off.append((o, C_lo + C_hi))
        o += C_lo + C_hi

    with tile.TileContext(nc) as tc:
        with (
            tc.tile_pool(name="const", bufs=1) as cpool,
            tc.tile_pool(name="meta", bufs=1) as mpool,
            tc.tile_pool(name="g", bufs=2) as gpool,
            tc.tile_pool(name="oh", bufs=6) as ohpool,
            tc.tile_pool(name="ys", bufs=2) as yspool,
            tc.tile_pool(name="yt", bufs=2) as ytpool,
            tc.tile_pool(name="os", bufs=2) as ospool,
            tc.tile_pool(name="py", bufs=1, space="PSUM") as pypool,
            tc.tile_pool(name="pt", bufs=2, space="PSUM") as ptpool,
            tc.tile_pool(name="po", bufs=2, space="PSUM") as popool,
        ):
            # constants
            iota_t = cpool.tile([128, 128], f32)
            nc.sync.dma_start(iota_t[:], iota_d.ap()[:])
            ident_t = cpool.tile([128, 128], f32)
            make_identity(nc, ident_t[:])
            coef_f32 = cpool.tile([64, 256], f32)
            nc.sync.dma_start(coef_f32[:], coef_d.ap()[:])
            coef_t = cpool.tile([64, 256], f32r)
            nc.vector.tensor_copy(coef_t[:], coef_f32[:])
            bias_t = cpool.tile([128, 1], f32)
            nc.sync.dma_start(bias_t[:], bias_d.ap()[:])
            idx_t = mpool.tile([128, TPC * STOT * 8], mybir.dt.int16)
            nc.sync.dma_start(idx_t[:], idx_d.ap()[:])
            rloc_t = mpool.tile([128, TPC * STOT], f32)
            nc.sync.dma_start(rloc_t[:], rloc_d.ap()[:])
            val_t = mpool.tile([128, TPC * STOT], f32)
            nc.sync.dma_start(val_t[:], val_d.ap()[:])

            yT = {}  # (k, b) -> staging tile [64, 256] across a tile pair
            qn = 0

            for t in range(TPC):
                mbase = t * STOT
                pair_off = (t % 2) * 128
                is_pair_start = t % 2 == 0
                is_orphan = t == TPC - 1 and is_pair_start

                g_t = gpool.tile([128, STOT * BC], f32r, tag="g")
                for opi, coff, nch, h in calls:
                    src = xg_d.ap()[SPLIT:, :] if h else xg_d.ap()[:SPLIT, :]
                    ib = (mbase + coff) * 8
                    nc.gpsimd.dma_gather(
                        out_ap=g_t[:, coff * BC:(coff + nch) * BC]
                        .rearrange("p (j f) -> p j f", f=BC),
                        in_ap=src,
                        idxs_ap=idx_t[:, ib:ib + nch * 8],
                        num_idxs=nch * 128,
                        num_idxs_reg=nch * 128,
                        elem_size=BC,
                        queue_num=qn % NQ,
                    )
                    qn += 1

                # identity features: dense rows of this core's xT slice
                ident_rows = yspool.tile([128, BC], f32, tag="yI")
                nc.sync.dma_start(
                    ident_rows[:], xTown_d.ap()[t * 128:(t + 1) * 128, :])

                # chunk matmuls per op
                y_sb = {"I": ident_rows}
                for opi, op in enumerate(OPNAMES):
                    coff, S_op = op_off[opi]
                    py_t = pypool.tile([128, BC], f32, tag=f"y{op}")
                    for j in range(S_op):
                        oh_t = ohpool.tile([128, 128], f32r, tag="oh")
                        mcol = mbase + coff + j
                        nc.vector.tensor_scalar(
                            out=oh_t[:],
                            in0=iota_t[:],
                            scalar1=rloc_t[:, mcol:mcol + 1],
                            scalar2=val_t[:, mcol:mcol + 1],
                            op0=mybir.AluOpType.is_equal,
                            op1=mybir.AluOpType.mult,
                        )
                        nc.tensor.matmul(
                            py_t[:],
                            oh_t[:],
                            g_t[:, (coff + j) * BC:(coff + j + 1) * BC],
                            start=(j == 0),
                            stop=(j == S_op - 1),
                        )
                    ys_t = yspool.tile([128, BC], f32, tag=f"ys{op}")
                    nc.scalar.activation(ys_t[:], py_t[:],
                                         mybir.ActivationFunctionType.Copy)
                    y_sb[op] = ys_t

                # transpose y[128r, 256bc] -> yT[(k,b)][64c, 128r]
                for ki, k in enumerate(["I", "L", "E", "N"]):
                    for b in range(B):
                        if is_pair_start:
                            yT[(k, b)] = ytpool.tile(
                                [64, 256], f32r, tag=f"yT{k}{b}",
                                name=f"yT{k}{b}_{t}")
                            if is_orphan:
                                nc.vector.memset(
                                    yT[(k, b)][:].bitcast(mybir.dt.float32),
                                    0.0)
                        pt_t = ptpool.tile([64, 128], f32, tag="psT")
                        nc.tensor.transpose(
                            pt_t[:], y_sb[k][:, b * 64:(b + 1) * 64],
                            ident_t[:])
                        nc.scalar.activation(
                            yT[(k, b)][:, pair_off:pair_off + 128], pt_t[:],
                            mybir.ActivationFunctionType.Copy)

                # coeffs matmuls on completed pair
                if not is_pair_start or is_orphan:
                    r0 = (t - 1 if not is_pair_start else t) * 128
                    ncols = 128 if is_orphan else 256
                    for b in range(B):
                        po_t = popool.tile([64, 256], f32, tag="po",
                                           name=f"po{b}_{t}")
                        for ki, k in enumerate(["I", "L", "E", "N"]):
                            nc.tensor.matmul(
                                po_t[:],
                                coef_t[:, ki * 64:(ki + 1) * 64],
                                yT[(k, b)][:],
                                start=(ki == 0),
                                stop=(ki == 3),
                            )
                        os_t = ospool.tile([64, 256], f32, tag="os",
                                           name=f"os{b}_{t}")
                        nc.vector.tensor_scalar(
                            out=os_t[:], in0=po_t[:],
                            scalar1=bias_t[0:64, :1], scalar2=None,
                            op0=mybir.AluOpType.add)
                        nc.sync.dma_start(
                            out_d.ap()[b:b + 1, :, r0:r0 + ncols]
                            .rearrange("b o r -> (b o) r"),
                            os_t[:, :ncols])

    nc.compile()
    return nc


def kernel(**inputs):
    x = np.asarray(inputs["x"], dtype=np.float32)
    coeffs = np.asarray(inputs["coeffs"], dtype=np.float32)
    bias = np.asarray(inputs["bias"], dtype=np.float32)

    xT = np.zeros((NVPAD, BC), np.float32)
    xT[:NV] = x.transpose(2, 0, 1).reshape(NV, BC)
    xg = _trunc_f32r(xT) if MM_MODE == "f32r" else xT

    ops = []
    for name in ("L", "EW", "NS"):
        S, idxP, rlocP, valP = _prep_op(
            inputs[f"{name}_row"], inputs[f"{name}_col"], inputs[f"{name}_val"])
        ops.append((S, idxP, rlocP, valP))
    S_ops = tuple(o[0] for o in ops)

    key = (S_ops, MM_MODE)
    if key not in _cache:
        _cache[key] = _build(S_ops)
    nc = _cache[key]

    iota = np.broadcast_to(np.arange(128, dtype=np.float32), (128, 128)).copy()
    coef_in = coeffs.transpose(1, 0, 2).reshape(64, 256).copy()  # [c, k*64+o]
    bias2 = np.tile(bias, 2).reshape(128, 1).astype(np.float32)

    in_maps = []
    for core in range(NCORES):
        t0, t1 = core * TPC, (core + 1) * TPC
        # idx16: per tile, per op: [128, C*8] wrapped-16 layout
        idx_parts = []
        for t in range(t0, t1):
            for o in ops:
                arr = o[1][t]  # [128, C] slot layout [p, j]: edge j*128+p
                flat = arr.transpose(1, 0).reshape(-1)  # [C*128] edge order
                idx_parts.append(_wrap16(flat))
        idx16 = np.concatenate(idx_parts, axis=1)
        rloc = np.concatenate(
            [np.concatenate([o[2][t] for o in ops], axis=1)
             for t in range(t0, t1)], axis=1)
        val = np.concatenate(
            [np.concatenate([o[3][t] for o in ops], axis=1)
             for t in range(t0, t1)], axis=1)
        in_maps.append({
            "xg": xg,
            "xTown": np.ascontiguousarray(xT[t0 * 128:t1 * 128]),
            "idx16": np.ascontiguousarray(idx16),
            "rloc": np.ascontiguousarray(rloc),
            "val": np.ascontiguousarray(val),
            "iota": iota, "coef": coef_in, "bias2": bias2,
        })

    res = run_bass_kernel_spmd(nc, in_maps, core_ids=list(range(NCORES)))
    out = np.concatenate([res.results[c]["out"] for c in range(NCORES)],
                         axis=2)
    return np.ascontiguousarray(out[:, :, :NV])
